# revision 27
# baseline (speedup 1.0000x reference)
"""Cached transformer encoder layer on 8 trn2 NeuronCores.

Sharding: data-parallel over (batch, query-block): core c handles batch
c//4, query tokens [(c%4)*512, (c%4+1)*512).  K/V recompute is replicated
per batch group (no collectives).  Softmax is permutation-invariant over
key positions, so keys are reordered as [recomputed | kept-cache] instead
of scattered into the cache.

Per-core pipeline:
  1. Q/K/V projections as fp8e4 DoubleRow matmuls (weights pre-scaled by
     64 on the host; 1/64 folded into the PSUM eviction).
  2. scores as bf16 matmuls, two heads packed per PE pass via
     tile_position (0,0)/(64,0) quadrant packing (K=64 each).
  3. exp on ACT over two-bank [128,1024] PSUM tiles, output fp8e4.
  4. attnV as fp8 DoubleRow (2 key-chunks per pass) with a ones-column in
     V giving the softmax denominator for free; normalization via one
     [2,SC] reciprocal per head pair + DRAM-roundtrip partition broadcast.
  5. Wo as fp8 DoubleRow over head-pair slabs (attn scaled by 32, Wo by
     64; 1/2048 folded into the eviction), residual + LN1.
  6. FFN in bf16 (fp8 fails the accuracy budget), relu fused into the
     PSUM eviction; W2 resident in SBUF with a query-chunk-outer loop so
     LN2/output DMA overlap the remaining FFN matmuls.
"""

import numpy as np
import ml_dtypes

import concourse.bass as bass
import concourse.mybir as mybir
import concourse.tile as tile
from concourse import bacc
from concourse.bass_interp import CoreSim
from concourse.bass_utils import run_bass_kernel_spmd
from concourse.masks import make_identity

F32 = mybir.dt.float32
BF16 = mybir.dt.bfloat16
FP8 = mybir.dt.float8e4
AF = mybir.ActivationFunctionType
OP = mybir.AluOpType
DR = mybir.MatmulPerfMode.DoubleRow

D = 1024
H = 16
HD = 64
DFF = 4096
S = 2048
B = 2
NCORES = 8
SC = 512          # query tokens per core
NKC = S // 128    # key chunks
NKP = NKC // 2    # key chunk pairs
EPS = 1e-5
WS = 64.0         # host-side weight scale for fp8 weights
AS = 32.0         # attnT scale before fp8 cast

_PROGRAM_CACHE: dict = {}


def build_program(R_u: int):
    """One SPMD program; every core runs it on its own input shards."""
    nc = bacc.Bacc("TRN2", target_bir_lowering=False, debug=False,
                   num_devices=NCORES)

    def din(name, shape, dt=F32):
        return nc.dram_tensor(name, shape, dt, kind="ExternalInput").ap()

    srcT8 = din("srcT8", [128, 4, 2, SC], FP8)     # [p, dpair, j, q]
    src_nat = din("src_nat", [4, 128, D])          # natural, q-chunked
    R_pad = ((R_u + 127) // 128) * 128
    srcRT8 = din("srcRT8", [128, 4, 2, R_pad], FP8)  # recompute rows (padded)
    kT_host = din("kT_host", [8, 128, S], BF16)    # per pair: 2 heads x hd
    v8_host = din("v8_host", [H, 128, NKP, 2, 128], FP8)
    Wq8_h = din("Wq8", [8, 128, 4, 2, 128], FP8)   # [pair, p, dpair, j, col]
    Wk8_h = din("Wk8", [8, 128, 4, 2, 128], FP8)
    Wv8_h = din("Wv8", [4, 128, 4, 2, 256], FP8)   # [quad, ...]
    Wo8_h = din("Wo8", [4, 128, 2, D], FP8)        # [gpair, p, g, col]
    W1_h = din("W1", [16, 128, 2, 8, 128], BF16)   # [mpair, p, mh, dchunk, col]
    W2_h = din("W2", [128, 32, D], BF16)           # [p, kchunk, col]
    bqT_h = din("bqT", [128, 8])
    bkT_h = din("bkT", [128, 8])
    b1T_h = din("b1T", [128, 32])
    pvec = din("pvec", [7, D])                     # bv,bo,b2,g1,beta1,g2,beta2
    out_d = nc.dram_tensor("out", [4, 128, D], F32, kind="ExternalOutput").ap()

    NRC = R_pad // 128                             # recompute row chunks

    with tile.TileContext(nc) as tc:
        with (
            tc.tile_pool(name="params", bufs=1) as params,
            tc.tile_pool(name="attn_keep", bufs=1) as akeep,
            tc.tile_pool(name="dram_scr", bufs=8, space="DRAM") as dscr,
        ):
            # --- constants / params ---
            ident = params.tile([128, 128], F32)
            make_identity(nc, ident[:])
            bqT = params.tile([128, 8], F32)
            bkT = params.tile([128, 8], F32)
            b1T = params.tile([128, 32], F32)
            nc.sync.dma_start(out=bqT[:], in_=bqT_h[:])
            nc.sync.dma_start(out=bkT[:], in_=bkT_h[:])
            pb = [params.tile([128, D], F32, name=f"pb{i}", tag=f"pb{i}")
                  for i in range(7)]
            nc.sync.dma_start(
                out=pb[0][:], in_=pvec[0:1, :].broadcast_to((128, D)))
            bv_b, bo_b, b2_b, g1_b, be1_b, g2_b, be2_b = pb
            eps_t = params.tile([128, 1], F32)
            nc.vector.memset(eps_t[:], EPS)

            attnT8 = akeep.tile([128, 8, SC], FP8)   # normalized attn * 32

            # ---------------- phase 1+2: projections + attention -----------
            with (
                tc.tile_pool(name="ph1", bufs=1) as ph1,
                tc.tile_pool(name="wst", bufs=3) as wst,
                tc.tile_pool(name="kv", bufs=2) as kvp,
                tc.tile_pool(name="qt", bufs=4) as qtp,
                tc.tile_pool(name="exp", bufs=2) as epool,
                tc.tile_pool(name="bc", bufs=3) as bcp,
                tc.tile_pool(name="evt", bufs=2) as evt,
                tc.tile_pool(name="ps_proj", bufs=2, space="PSUM") as psP,
                tc.tile_pool(name="ps_s", bufs=2, space="PSUM") as psS,
                tc.tile_pool(name="ps_at", bufs=2, space="PSUM") as psA,
            ):
                srcT_sb = ph1.tile([128, 4, 2, SC], FP8)
                srcRT_sb = ph1.tile([128, 4, 2, R_pad], FP8)
                nc.sync.dma_start(out=srcT_sb[:], in_=srcT8[:])
                nc.sync.dma_start(out=srcRT_sb[:], in_=srcRT8[:])
                attnU = ph1.tile([128, 8, SC], BF16)  # unnormalized numerators
                den_i = [ph1.tile([1, SC], BF16, name=f"den_i{h}",
                                  tag=f"dh{h}") for h in range(16)]

                for quad in range(4):
                    pairs = (2 * quad, 2 * quad + 1)
                    den4 = bcp.tile([4, SC], BF16, name=f"den4_{quad}",
                                    tag="dn4", bufs=2)
                    kT_t = {}
                    qT_t = {}
                    v_t = {}
                    for g in pairs:
                        kT_t[g] = kvp.tile([128, S], BF16, name=f"kT{g}",
                                           tag="kT", bufs=4)
                        nc.sync.dma_start(out=kT_t[g][:], in_=kT_host[g])
                        for j in range(2):
                            h = 2 * g + j
                            v_t[h] = kvp.tile([128, NKP, 2, 128], FP8,
                                              name=f"v{h}", tag="v", bufs=8)
                            nc.sync.dma_start(out=v_t[h][:], in_=v8_host[h])

                    # Q and K projections per pair (fp8 DoubleRow over dpairs)
                    for g in pairs:
                        wq = wst.tile([128, 4, 2, 128], FP8, tag="w", bufs=3)
                        nc.sync.dma_start(out=wq[:], in_=Wq8_h[g])
                        psq = psP.tile([128, SC], F32, tag="pp")
                        for dp in range(4):
                            nc.tensor.matmul(psq[:], wq[:, dp], srcT_sb[:, dp],
                                             start=(dp == 0), stop=(dp == 3),
                                             perf_mode=DR)
                        qT_t[g] = qtp.tile([128, SC], BF16, name=f"qT{g}",
                                           tag="qT")
                        nc.vector.tensor_scalar(
                            out=qT_t[g][:], in0=psq[:], scalar1=1.0 / WS,
                            scalar2=bqT[:, g:g + 1], op0=OP.mult, op1=OP.add)

                        wk = wst.tile([128, 4, 2, 128], FP8, tag="w", bufs=3)
                        nc.sync.dma_start(out=wk[:], in_=Wk8_h[g])
                        psk = psP.tile([128, R_pad], F32, tag="pp")
                        for dp in range(4):
                            nc.tensor.matmul(psk[:], wk[:, dp],
                                             srcRT_sb[:, dp],
                                             start=(dp == 0), stop=(dp == 3),
                                             perf_mode=DR)
                        nc.vector.tensor_scalar(
                            out=kT_t[g][:, 0:R_u], in0=psk[:, 0:R_u],
                            scalar1=1.0 / WS,
                            scalar2=bkT[:, g:g + 1], op0=OP.mult, op1=OP.add)

                    # V projection for the whole quad (256 head dims)
                    wv = wst.tile([128, 4, 2, 256], FP8, tag="wv", bufs=2)
                    nc.sync.dma_start(out=wv[:], in_=Wv8_h[quad])
                    for rc in range(NRC):
                        nr = min(128, R_u - rc * 128)
                        psv = psP.tile([128, 256], F32, name=f"psv{rc}",
                                       tag="pp")
                        for dp in range(4):
                            nc.tensor.matmul(
                                psv[:], srcRT_sb[:, dp, :,
                                                 rc * 128:(rc + 1) * 128],
                                wv[:, dp], start=(dp == 0), stop=(dp == 3),
                                perf_mode=DR)
                        vstage = evt.tile([128, 256], F32, tag="vs")
                        nc.vector.scalar_tensor_tensor(
                            out=vstage[:nr, :], in0=psv[:nr, :],
                            scalar=1.0 / WS,
                            in1=bv_b[:nr, quad * 256:(quad + 1) * 256],
                            op0=OP.mult, op1=OP.add)
                        for jj in range(4):
                            h = 4 * quad + jj
                            nc.vector.tensor_copy(
                                v_t[h][0:nr, rc // 2, rc % 2, 0:HD],
                                vstage[0:nr, jj * 64:(jj + 1) * 64])

                    # attention for both pairs of the quad
                    for g in pairs:
                        at_ps = [psA.tile([128, SC], F32, name=f"at{jj}",
                                          tag="at") for jj in range(2)]
                        et8 = [epool.tile([128, NKP, 2, SC], FP8,
                                          name=f"et{jj}", tag="e")
                               for jj in range(2)]
                        for kp in range(NKP):
                            ssc = [psS.tile([128, 1024], F32,
                                            name=f"ssc{jj}", tag="sc")
                                   for jj in range(2)]
                            for half in range(2):
                                kc = 2 * kp + half
                                ks = slice(kc * 128, (kc + 1) * 128)
                                cs = slice(half * 512, half * 512 + 512)
                                nc.tensor.matmul(
                                    ssc[0][:, cs], kT_t[g][0:64, ks],
                                    qT_t[g][0:64, :], start=True, stop=True,
                                    tile_position=(0, 0))
                                nc.tensor.matmul(
                                    ssc[1][:, cs], kT_t[g][64:128, ks],
                                    qT_t[g][64:128, :], start=True, stop=True,
                                    tile_position=(64, 0))
                            for j in range(2):
                                nc.scalar.activation(
                                    et8[j][:, kp], ssc[j][:], AF.Exp,
                                    scale=0.125)
                            for j in range(2):
                                h = 2 * g + j
                                nc.tensor.matmul(
                                    at_ps[j][:], v_t[h][:, kp],
                                    et8[j][:, kp], start=(kp == 0),
                                    stop=(kp == NKP - 1), perf_mode=DR)
                        # stash denominators + unnormalized numerators;
                        # normalization happens once after all pairs.
                        gi = g - 2 * quad
                        for j in range(2):
                            h = 2 * g + j
                            nc.vector.tensor_copy(den_i[h][:],
                                                  at_ps[j][64:65, :])
                            nc.sync.dma_start(
                                out=den4[2 * gi + j:2 * gi + j + 1, :],
                                in_=den_i[h][:])
                            nc.vector.tensor_copy(
                                attnU[j * 64:j * 64 + 64, g, :],
                                at_ps[j][0:64, :])

                    # ---- per-quad normalize: recip + broadcast + mults --
                    rr4 = bcp.tile([4, SC], BF16, name=f"rr4_{quad}",
                                   tag="rr4", bufs=2)
                    with nc.allow_low_precision(
                            reason="softmax denominators are O(2048); bf16 "
                                   "reciprocal adds ~0.4% on a tiny term"):
                        nc.vector.reciprocal(rr4[:], den4[:])
                    dden4 = dscr.tile([2, 2, SC], BF16,
                                      name=f"dden4_{quad}", tag="dd4")
                    nc.sync.dma_start(out=dden4[:].transpose((1, 0, 2)),
                                      in_=rr4[:])
                    rec4 = bcp.tile([128, 2, SC], BF16, name=f"rec4_{quad}",
                                    tag="bc4", bufs=2)
                    for j in range(2):
                        nc.sync.dma_start(
                            out=rec4[j * 64:j * 64 + 64, :, :],
                            in_=dden4[j].unsqueeze(0).broadcast_to(
                                (64, 2, SC)))
                    for gi2, g in enumerate(pairs):
                        nc.vector.scalar_tensor_tensor(
                            out=attnT8[:, g, :], in0=attnU[:, g, :],
                            scalar=AS, in1=rec4[:, gi2, :],
                            op0=OP.mult, op1=OP.mult)

                # load LN/FFN params during the attention tail
                nc.sync.dma_start(out=b1T[:], in_=b1T_h[:])
                for i in range(1, 7):
                    nc.sync.dma_start(
                        out=pb[i][:],
                        in_=pvec[i:i + 1, :].broadcast_to((128, D)))

            # ---------------- phase 3: Wo + residual + LN1 -----------------
            with (
                tc.tile_pool(name="ffn_keep", bufs=1) as fkeep,
                tc.tile_pool(name="lnt", bufs=2) as lnt,
                tc.tile_pool(name="lns", bufs=8) as lns,
            ):
                x1 = fkeep.tile([128, 4, D], F32)
                x1T = fkeep.tile([128, 8, SC], BF16)
                wo_ln_scope = (
                    tc.tile_pool(name="wst3", bufs=1),
                    tc.tile_pool(name="ps_o", bufs=4, space="PSUM"),
                    tc.tile_pool(name="ps_t", bufs=2, space="PSUM"),
                )
                wst3 = wo_ln_scope[0].__enter__()
                psO = wo_ln_scope[1].__enter__()
                psT = wo_ln_scope[2].__enter__()

                wo_t = [wst3.tile([128, 2, D], FP8, name=f"wo{gp}",
                                  tag="w3", bufs=4) for gp in range(4)]
                for gp in range(4):
                    nc.sync.dma_start(out=wo_t[gp][:], in_=Wo8_h[gp])
                pso = {}
                for qc in range(4):
                    qs = slice(qc * 128, (qc + 1) * 128)
                    for half in range(2):
                        pso[2 * qc + half] = psO.tile(
                            [128, 512], F32, name=f"pso{2*qc+half}", tag="po")
                        hs = slice(half * 512, half * 512 + 512)
                        for gp in range(4):
                            nc.tensor.matmul(
                                pso[2 * qc + half][:],
                                attnT8[:, 2 * gp:2 * gp + 2, qs],
                                wo_t[gp][:, :, hs],
                                start=(gp == 0), stop=(gp == 3), perf_mode=DR)

                def layer_norm(out_ap, x_ap, gamma, beta):
                    st = lns.tile([128, 2, 6], F32, tag="st")
                    nc.vector.bn_stats(st[:, 0, :], x_ap[:, 0:512])
                    nc.vector.bn_stats(st[:, 1, :], x_ap[:, 512:1024])
                    mv = lns.tile([128, 2], F32, tag="mv")
                    nc.vector.bn_aggr(mv[:], st[:])
                    sd = lns.tile([128, 1], F32, tag="s")
                    nc.scalar.activation(sd[:], mv[:, 1:2], AF.Sqrt,
                                         bias=eps_t[:], scale=1.0)
                    rs = lns.tile([128, 1], F32, tag="s")
                    nc.vector.reciprocal(rs[:], sd[:])
                    xh = lnt.tile([128, D], F32, tag="xm")
                    nc.vector.tensor_scalar(out=xh[:], in0=x_ap,
                                            scalar1=mv[:, 0:1], scalar2=rs[:],
                                            op0=OP.subtract, op1=OP.mult)
                    nc.vector.scalar_tensor_tensor(
                        out=out_ap, in0=xh[:], scalar=1.0, in1=gamma,
                        op0=OP.mult, op1=OP.mult)
                    nc.vector.tensor_tensor(out=out_ap, in0=out_ap, in1=beta,
                                            op=OP.add)

                for qc in range(4):
                    ssb = lnt.tile([128, D], F32, tag="src")
                    nc.sync.dma_start(out=ssb[:], in_=src_nat[qc])
                    xr = lnt.tile([128, D], F32, tag="xr")
                    for half in range(2):
                        hs = slice(half * 512, half * 512 + 512)
                        nc.vector.scalar_tensor_tensor(
                            out=xr[:, hs], in0=pso[2 * qc + half][:],
                            scalar=1.0 / (WS * AS), in1=ssb[:, hs],
                            op0=OP.mult, op1=OP.add)
                    layer_norm(x1[:, qc, :], xr[:], g1_b[:], be1_b[:])
                    for dc in range(8):
                        pst = psT.tile([128, 128], F32, tag="pt")
                        nc.tensor.transpose(
                            pst[:], x1[:, qc, dc * 128:(dc + 1) * 128],
                            ident[:])
                        nc.scalar.copy(
                            x1T[:, dc, qc * 128:(qc + 1) * 128], pst[:])

                wo_ln_scope[2].__exit__(None, None, None)
                wo_ln_scope[1].__exit__(None, None, None)
                wo_ln_scope[0].__exit__(None, None, None)

                # ---------------- phase 4: FFN + residual + LN2 -------------
                with (
                    tc.tile_pool(name="ff1keep", bufs=1) as f1k,
                    tc.tile_pool(name="wst4", bufs=3) as wst4,
                ):
                    ff1T = f1k.tile([128, 32, SC], BF16)
                    w2_all = f1k.tile([128, 32, D], BF16)
                    for k in range(0, 32, 8):
                        nc.sync.dma_start(out=w2_all[:, k:k + 8, :],
                                          in_=W2_h[:, k:k + 8, :])
                    psf1_cm = tc.tile_pool(name="ps_f1", bufs=3, space="PSUM")
                    psF1 = psf1_cm.__enter__()
                    for mp in range(16):
                        w1 = wst4.tile([128, 2, 8, 128], BF16, tag="w1")
                        nc.sync.dma_start(out=w1[:], in_=W1_h[mp])
                        psf = psF1.tile([128, 1024], F32, tag="pf")
                        for mh in range(2):
                            m = 2 * mp + mh
                            cs = slice(mh * 512, mh * 512 + 512)
                            for d in range(8):
                                nc.tensor.matmul(
                                    psf[:, cs], w1[:, mh, d, :], x1T[:, d, :],
                                    start=(d == 0), stop=(d == 7))
                        # relu + bias, evict both halves in one ACT pass each
                        for mh in range(2):
                            m = 2 * mp + mh
                            cs = slice(mh * 512, mh * 512 + 512)
                            nc.scalar.activation(ff1T[:, m, :], psf[:, cs],
                                                 AF.Relu, bias=b1T[:, m:m + 1])

                    psf1_cm.__exit__(None, None, None)
                    with tc.tile_pool(name="ps_f2", bufs=4,
                                      space="PSUM") as psF2:
                        for qc in range(4):
                            qs = slice(qc * 128, (qc + 1) * 128)
                            psf2 = [psF2.tile([128, 512], F32,
                                              name=f"psf2_{qc}_{i}",
                                              tag="p2")
                                    for i in range(2)]
                            for k in range(32):
                                for half in range(2):
                                    hs = slice(half * 512, half * 512 + 512)
                                    nc.tensor.matmul(
                                        psf2[half][:],
                                        ff1T[:, k, qs], w2_all[:, k, hs],
                                        start=(k == 0), stop=(k == 31))
                            xr2 = lnt.tile([128, D], F32, tag="xr")
                            for half in range(2):
                                hs = slice(half * 512, half * 512 + 512)
                                nc.vector.tensor_tensor(
                                    out=xr2[:, hs], in0=psf2[half][:],
                                    in1=x1[:, qc, hs], op=OP.add)
                            nc.vector.tensor_tensor(out=xr2[:], in0=xr2[:],
                                                    in1=b2_b[:], op=OP.add)
                            xo = lnt.tile([128, D], F32, tag="xo")
                            layer_norm(xo[:], xr2[:], g2_b[:], be2_b[:])
                            nc.sync.dma_start(out=out_d[qc], in_=xo[:])

    nc.finalize()
    # A (cheap, no-exec) CoreSim pass is required before the first HW run.
    sim = CoreSim(nc, no_exec=True)
    sim.simulate(check_with_hw=False)
    return nc


def shard_inputs(inputs: dict, R_u: int, idx_u: np.ndarray,
                 keep: np.ndarray) -> list:
    """Build the 8 per-core input maps (host-side layout prep only)."""
    bf16 = ml_dtypes.bfloat16
    fp8 = ml_dtypes.float8_e4m3
    f = lambda a: np.ascontiguousarray(np.asarray(a), dtype=np.float32)
    src = f(inputs["src"])
    cached_k = f(inputs["cached_k"])
    cached_v = f(inputs["cached_v"])

    def w8_pair(W, cols, ngrp):
        # (W*WS).T [D, D'] -> [ngrp, 128, 4 dpair, 2 j, cols]
        return np.ascontiguousarray(
            (W * WS).T.reshape(4, 2, 128, ngrp, cols)
            .transpose(3, 2, 0, 1, 4)).astype(fp8)

    shared = {
        "Wq8": w8_pair(f(inputs["Wq"]), 128, 8),
        "Wk8": w8_pair(f(inputs["Wk"]), 128, 8),
        "Wv8": w8_pair(f(inputs["Wv"]), 256, 4),
        "Wo8": np.ascontiguousarray(
            (f(inputs["Wo"]) * WS).T.reshape(4, 2, 128, D)
            .transpose(0, 2, 1, 3)).astype(fp8),
        "W1": np.ascontiguousarray(
            f(inputs["W1"]).T.reshape(8, 128, 16, 2, 128)
            .transpose(2, 1, 3, 0, 4)).astype(bf16),
        "W2": np.ascontiguousarray(
            f(inputs["W2"]).T.reshape(32, 128, D)
            .transpose(1, 0, 2)).astype(bf16),
        "bqT": np.ascontiguousarray(f(inputs["bq"]).reshape(8, 128).T),
        "bkT": np.ascontiguousarray(f(inputs["bk"]).reshape(8, 128).T),
        "b1T": np.ascontiguousarray(f(inputs["b1"]).reshape(32, 128).T),
        "pvec": np.stack([f(inputs[k]) for k in
                          ("bv", "bo", "b2", "g1", "beta1", "g2", "beta2")]),
    }

    per_b = {}
    for b in range(B):
        kT_full = np.zeros((H, HD, S), np.float32)
        kT_full[:, :, R_u:] = cached_k[b][:, keep, :].transpose(0, 2, 1)
        kT_host = np.ascontiguousarray(kT_full.reshape(8, 128, S)).astype(bf16)
        vfull = np.zeros((H, S, 128), np.float32)
        vfull[:, :, HD] = 1.0
        vfull[:, R_u:, 0:HD] = cached_v[b][:, keep, :]
        v8_host = np.ascontiguousarray(
            vfull.reshape(H, NKP, 2, 128, 128)
            .transpose(0, 3, 1, 2, 4)).astype(fp8)
        R_pad = ((R_u + 127) // 128) * 128
        src_rec = np.zeros((R_pad, D), np.float32)
        src_rec[0:R_u] = src[b][idx_u, :]
        srcRT8 = np.ascontiguousarray(
            src_rec.T.reshape(4, 2, 128, R_pad).transpose(2, 0, 1, 3)
        ).astype(fp8)
        per_b[b] = (kT_host, v8_host, srcRT8)

    in_maps = []
    for c in range(NCORES):
        b, tb = c // 4, c % 4
        sl = src[b, tb * SC:(tb + 1) * SC, :]          # [SC, D]
        kT_host, v8_host, srcRT8 = per_b[b]
        m = dict(shared)
        m["srcT8"] = np.ascontiguousarray(
            sl.T.reshape(4, 2, 128, SC).transpose(2, 0, 1, 3)).astype(fp8)
        m["src_nat"] = np.ascontiguousarray(
            (sl + f(inputs["bo"])[None, :]).reshape(4, 128, D))
        m["srcRT8"] = srcRT8
        m["kT_host"] = kT_host
        m["v8_host"] = v8_host
        in_maps.append(m)
    return in_maps


def kernel(**inputs) -> np.ndarray:
    idx = np.asarray(inputs["recompute_idx"]).astype(np.int64)
    idx_u = np.unique(idx)
    R_u = int(idx_u.shape[0])
    keep = np.setdiff1d(np.arange(S, dtype=np.int64), idx_u)

    if R_u not in _PROGRAM_CACHE:
        _PROGRAM_CACHE[R_u] = build_program(R_u)
    nc = _PROGRAM_CACHE[R_u]

    in_maps = shard_inputs(inputs, R_u, idx_u, keep)
    res = run_bass_kernel_spmd(nc, in_maps, list(range(NCORES)))
    out = np.empty((B, S, D), np.float32)
    for c in range(NCORES):
        b, tb = c // 4, c % 4
        out[b, tb * SC:(tb + 1) * SC, :] = \
            res.results[c]["out"].reshape(SC, D)
    return out
